# revision 45
# baseline (speedup 1.0000x reference)
"""Cross-modal attention kernel for Trainium2 -- data-parallel over batch on 8 cores.

Reference computation per sample (C=256, H=W=64, N=H*W=4096, dqk=32):
    q = Wq @ x + bq; k = Wk @ y + bk; v = Wv @ y + bv
    out = gamma * (v @ softmax_j(q^T k)^T) + x

The Act engine's exp over the full 4096x4096 energy matrix is the binding
resource (~133us busy at 0.833ns/col on [128, free] tiles, dtype-independent),
so the kernel is organized as one continuous act stream with everything else
scheduled underneath it:

  - No separate projection phase: q/k/v projections are interleaved into the
    attention pipeline's PE-queue slack as deadline-ordered "filler" work.
  - Wq/Wk are loaded pre-replicated 4x along the output dim, so one matmul
    per channel-chunk directly yields q/k in the 4-row-group layout that the
    4-way tile_position energy packing wants (no broadcast copies at all).
  - Energy is computed TRANSPOSED (E^T[j,i], keys on partitions); exp is
    applied unnormalized to fp8e4m3; AV and the softmax denominator are
    MatmulPerfMode.DoubleRow fp8 contractions (2 j-tiles per pass); the
    denominator's normalization happens on the [C, IBLK] output.
  - The exp act table is preloaded at t=0 by a dummy 1-element activation;
    a burst of warmup matmuls keeps PE continuously busy from t~0.4us so it
    reaches full pstate before the first projection.
  - AV runs at lag 3 behind the energy groups and the last three AV groups
    of each block spill into the next block's first three slots, so the PE
    queue always has ready work and its idle gaps stay below the pstate
    reset threshold.
  - The residual x is re-read from the f32r SBUF projection operand via
    bitcast -- x is DMA'd once, not twice.
  - Tail (1/den, *gamma, +x, store) is split DVE/Pool so the two channel
    chunks normalize in parallel; within each AV pair the den matmul issues
    first so the final block's tail starts as early as possible.

PSUM budget (8 banks of 2KB): energy ring 2x[128,1024]f32 = 4, av accums
2x[128,512] = 2, den 1, projection scratch [128,512] = 1.
"""

import sys

if "/opt/trn_rl_repo" not in sys.path:
    sys.path.insert(0, "/opt/trn_rl_repo")

import numpy as np

import concourse.bacc as bacc
import concourse.mybir as mybir
import concourse.tile as tile
from concourse.bass_utils import run_bass_kernel_spmd

F32 = mybir.dt.float32
F32R = mybir.dt.float32r
BF16 = mybir.dt.bfloat16
FP8 = mybir.dt.float8e4

B, C, HW, D = 8, 256, 4096, 32
CH = C // 128
IBLK = 512
NIB = HW // IBLK
NJT = HW // 128
NPAIR = NJT // 2
NG = NJT // 4
NWARM = 12
LAG = 6
EXPF = mybir.ActivationFunctionType.Exp
MULT = mybir.AluOpType.mult
ADD = mybir.AluOpType.add
DROW = mybir.MatmulPerfMode.DoubleRow


def _build():
    nc = bacc.Bacc("TRN2", target_bir_lowering=False, debug=False, num_devices=8)

    xr = nc.dram_tensor("xr", [C, HW], F32R, kind="ExternalInput")
    yr = nc.dram_tensor("yr", [C, HW], F32R, kind="ExternalInput")
    wq4d = nc.dram_tensor("wq4", [C, 128], F32R, kind="ExternalInput")
    wk4d = nc.dram_tensor("wk4", [C, 128], F32R, kind="ExternalInput")
    wvd = nc.dram_tensor("wvT", [C, C], F32R, kind="ExternalInput")
    bq4d = nc.dram_tensor("bq4", [128, 1], F32, kind="ExternalInput")
    bk4d = nc.dram_tensor("bk4", [128, 1], F32, kind="ExternalInput")
    gmd = nc.dram_tensor("gmd", [128, 1], F32, kind="ExternalInput")
    out = nc.dram_tensor("out", [C, HW], F32, kind="ExternalOutput")

    tc = tile.TileContext(nc)
    with tc:
        with (
            tc.tile_pool(name="cst", bufs=1) as cst,
            tc.tile_pool(name="io", bufs=1) as io,
            tc.tile_pool(name="qkv", bufs=1) as qkv,
        ):
            wq4_sb = cst.tile([128, CH * 128], F32R)
            wk4_sb = cst.tile([128, CH * 128], F32R)
            wv_sb = cst.tile([128, CH * C], F32R)
            bq4_sb = cst.tile([128, 1], F32)
            bk4_sb = cst.tile([128, 1], F32)
            gm_sb = cst.tile([128, 1], F32)
            ones_sb = cst.tile([128, 2 * 128], FP8)
            scr = cst.tile([1, 2], F32)

            xr_sb = io.tile([128, CH * HW], F32R)
            yr_sb = io.tile([128, CH * HW], F32R)
            q4 = qkv.tile([128, HW], BF16)
            k4 = qkv.tile([128, HW], BF16)
            vt = qkv.tile([128, NJT * C], FP8)

            # exp act-table preload: first act-engine instruction, no deps
            # beyond the memset, so LoadActFuncSet runs at t~0.  memset on
            # Pool (95ns) so the PE warmup below can start immediately.
            nc.gpsimd.memset(ones_sb[:], 1.0)
            nc.scalar.activation(scr[:, 0:1], ones_sb[0:1, 0:1], EXPF)

            # weights + small constants via the gpsimd SWDGE queue: its
            # descriptor generation runs in parallel with the HWDGE input
            # stream, and the transfers are small enough to slip between
            # the big input transfers on the DMA engines
            nc.gpsimd.dma_start(
                wk4_sb[:].rearrange("P (ch c) -> P ch c", ch=CH),
                wk4d.rearrange("(ch p) c -> p ch c", ch=CH),
            )
            nc.gpsimd.dma_start(bk4_sb[:], bk4d[:])
            nc.gpsimd.dma_start(
                wv_sb[:].rearrange("P (ch c) -> P ch c", ch=CH),
                wvd.rearrange("(ch p) c -> p ch c", ch=CH),
            )
            nc.gpsimd.dma_start(gm_sb[:], gmd[:])

            # inputs on the sync/HWDGE queue in dependency-deadline order:
            # the first energy group needs wk4+y0 then wq4+x0; y chunk c
            # feeds k-chunk c (needed at block-0 slot c) and vt chunk c.
            yr3 = yr.rearrange("(ch p) N -> p ch N", ch=CH)
            xr3 = xr.rearrange("(ch p) N -> p ch N", ch=CH)
            yr_sb3 = yr_sb[:].rearrange("P (ch N) -> P ch N", ch=CH)
            xr_sb3 = xr_sb[:].rearrange("P (ch N) -> P ch N", ch=CH)

            def ld(dst3, src3, c0, c1):
                nc.sync.dma_start(dst3[:, :, c0:c1], src3[:, :, c0:c1])

            ld(yr_sb3, yr3, 0, IBLK)
            nc.sync.dma_start(wq4_sb[:, 0:128], wq4d[0:128, :])
            nc.sync.dma_start(wq4_sb[:, 128:256], wq4d[128:256, :])
            # x chunk 0 split per channel-chunk so q0's two matmuls can
            # pipeline with the two transfers; bq4 after x0 (only the
            # bias-add needs it)
            nc.sync.dma_start(xr_sb[:, 0:IBLK], xr[0:128, 0:IBLK])
            nc.sync.dma_start(xr_sb[:, HW:HW + IBLK], xr[128:256, 0:IBLK])
            nc.sync.dma_start(bq4_sb[:], bq4d[:])
            for c in range(1, NIB):
                ld(yr_sb3, yr3, c * IBLK, (c + 1) * IBLK)
            for n in range(1, NIB):
                ld(xr_sb3, xr3, n * IBLK, (n + 1) * IBLK)

            with (
                tc.tile_pool(name="ptp", bufs=2) as ptp,
                tc.tile_pool(name="wrk", bufs=2) as wrk,
                tc.tile_pool(name="psE", bufs=1, space="PSUM") as psE,
                tc.tile_pool(name="psAV", bufs=1, space="PSUM") as psAV,
                tc.tile_pool(name="psPR", bufs=1, space="PSUM") as psPR,
            ):
                psa = psPR.tile([128, IBLK], F32, name="psa")
                vrr = [0]
                kln = [0]
                loan = [True]

                def vt_region():
                    # scratch bank for a vt j-tile PAIR.  While the av/den
                    # psum banks are still unused (before block 0's first av
                    # accumulation) they are loaned out for a depth-3
                    # rotation; afterwards the single psa bank serves the
                    # few remaining pairs.
                    if loan[0]:
                        r = vrr[0] & 1
                        vrr[0] += 1
                        if r == 0:
                            return psa[:]
                        t = psAV.tile([128, IBLK], F32,
                                      name=f"vln_{vrr[0]}", tag="av0")
                        return t[:]
                    return psa[:]

                def kq_region():
                    if loan[0]:
                        kln[0] += 1
                        t = psAV.tile([128, IBLK], F32,
                                      name=f"kln_{kln[0]}",
                                      tag="den" if kln[0] & 1 else "av1")
                        return t[:]
                    return psa[:]

                def k_chunk(c, w_sb=None, b_sb=None, src=None, dst=None):
                    w_sb = wk4_sb if w_sb is None else w_sb
                    b_sb = bk4_sb if b_sb is None else b_sb
                    src = yr_sb if src is None else src
                    dst = k4 if dst is None else dst
                    ps = kq_region()
                    for h in range(CH):
                        nc.tensor.matmul(
                            ps,
                            w_sb[:, h * 128:(h + 1) * 128],
                            src[:, h * HW + c * IBLK: h * HW + (c + 1) * IBLK],
                            start=(h == 0),
                            stop=(h == CH - 1),
                        )
                    nc.vector.tensor_scalar_add(
                        dst[:, c * IBLK:(c + 1) * IBLK], ps, b_sb[:, 0:1]
                    )

                def q_block(n):
                    k_chunk(n, wq4_sb, bq4_sb, xr_sb, q4)

                def vt_pair(p):
                    # project two j-tiles into one scratch bank, drain with
                    # a single [128,512] DVE copy (Pool cannot read PSUM on
                    # real hw, so all drains share DVE -- pairing halves the
                    # per-tile drain cost)
                    ps = vt_region()
                    for s in range(2):
                        jt = 2 * p + s
                        for h in range(CH):
                            nc.tensor.matmul(
                                ps[:, s * C:(s + 1) * C],
                                yr_sb[:, h * HW + jt * 128: h * HW + (jt + 1) * 128],
                                wv_sb[:, h * C:(h + 1) * C],
                                start=(h == 0),
                                stop=(h == CH - 1),
                            )
                    nc.vector.tensor_copy(vt[:, 2 * p * C:(2 * p + 2) * C], ps[:])

                # deadline-ordered projection filler, split into vt tiles
                # (interleaved between other PE work so the psum scratch
                # ping-pong latency hides) and k/q chunks.
                fill_vt = {}
                fill_kq = {}
                for g in range(6):
                    # loan window: 2 vt pairs per slot on the loaned scratch
                    fill_vt[(0, g)] = [2 * g, 2 * g + 1]
                fill_vt[(1, 0)] = [14]
                fill_vt[(1, 1)] = [15]
                fill_vt[(1, 2)] = [12]
                fill_vt[(1, 3)] = [13]
                for g in range(1, 8):
                    fill_kq[(0, g - 1)] = lambda g=g: k_chunk(g)
                fill_kq[(0, 7)] = lambda: q_block(1)
                fill_kq[(1, 5)] = lambda: q_block(2)
                for n in range(2, 7):
                    fill_kq[(n, 1)] = lambda n=n: q_block(n + 1)

                def et_group(n, g, pt):
                    # energy for (i-block n, group g): 4 row-packed K=32
                    # matmuls into two 2-bank psum tiles, then exp into pt
                    ets = [
                        psE.tile([128, 2 * IBLK], F32,
                                 name=f"et{h}_{n}_{g}", tag="et", bufs=2)
                        for h in range(2)
                    ]
                    for q in range(4):
                        jt = 4 * g + q
                        nc.tensor.matmul(
                            ets[q // 2][:, (q % 2) * IBLK:(q % 2 + 1) * IBLK],
                            k4[32 * q:32 * (q + 1), jt * 128:(jt + 1) * 128],
                            q4[32 * q:32 * (q + 1), n * IBLK:(n + 1) * IBLK],
                            start=True,
                            stop=True,
                            tile_position=(32 * q, 0),
                        )
                    for h in range(2):
                        nc.scalar.activation(
                            pt[:, (4 * g + 2 * h) * IBLK:(4 * g + 2 * h + 2) * IBLK],
                            ets[h][:], EXPF,
                        )

                ones_pair = ones_sb[:].rearrange("P (s c) -> P s c", s=2)

                def make_tail(n, get_avden, last=False):
                    def tail():
                        av, den = get_avden()
                        rgb = wrk.tile([128, IBLK], F32,
                                       name=f"rgb_{n}", tag="rgb")
                        nc.vector.reciprocal(rgb[:], den[:])
                        ot = wrk.tile([128, CH * IBLK], F32, name=f"ot_{n}", tag="ot")
                        # gamma folds into the scalar slot of the stt, so the
                        # whole tail is reciprocal + 2 ops per channel (all
                        # DVE: av is PSUM, which only DVE can read)
                        for ch in range(CH):
                            xres = xr_sb[
                                :, ch * HW + n * IBLK: ch * HW + (n + 1) * IBLK
                            ].bitcast(F32)
                            tmp = wrk.tile([128, IBLK], F32,
                                           name=f"tmp_{n}_{ch}", tag=f"tmp{ch}")
                            nc.vector.scalar_tensor_tensor(
                                tmp[:], av[ch][:], gm_sb[:, 0:1], rgb[:],
                                MULT, MULT,
                            )
                            # the final +x is SBUF-only, so ch0's can run on
                            # Pool while DVE continues with ch1's stt
                            eng = nc.gpsimd if ch == 0 else nc.vector
                            eng.tensor_tensor(
                                ot[:, ch * IBLK:(ch + 1) * IBLK],
                                tmp[:], xres, ADD,
                            )
                            # per-channel store so ch0 ships while ch1 runs
                            nc.sync.dma_start(
                                out[ch * 128:(ch + 1) * 128,
                                    n * IBLK:(n + 1) * IBLK],
                                ot[:, ch * IBLK:(ch + 1) * IBLK],
                            )
                    return tail

                prev_av = None
                prev_tail = None
                for n in range(NIB):
                    pt = ptp.tile([128, NJT * IBLK], FP8, name=f"pt_{n}", tag="pt")

                    # av/den accumulators are allocated lazily at the first
                    # av_pair so block 0's loan tiles (same tags) precede
                    # them in the ring's WAR chain
                    holder = {}

                    def get_avden(n=n, holder=holder):
                        if "av" not in holder:
                            holder["av"] = [
                                psAV.tile([128, IBLK], F32,
                                          name=f"av{ch}_{n}", tag=f"av{ch}")
                                for ch in range(CH)
                            ]
                            holder["den"] = psAV.tile(
                                [128, IBLK], F32, name=f"den_{n}", tag="den")
                        return holder["av"], holder["den"]

                    def av_pair(p, pt=pt, get=get_avden):
                        av, den = get()
                        # DoubleRow AV + denominator for j-tile pair p:
                        # virtual K=256 contracts two j-tiles at once.  den
                        # first so the last block's tail can start before
                        # its final av matmuls retire.
                        ptp_ap = pt[:, 2 * p * IBLK:(2 * p + 2) * IBLK].rearrange(
                            "P (s N) -> P s N", s=2
                        )
                        vtp_ap = vt[:, 2 * p * C:(2 * p + 2) * C].rearrange(
                            "P (s c) -> P s c", s=2
                        )
                        nc.tensor.matmul(
                            den[:],
                            ones_pair,
                            ptp_ap,
                            start=(p == 0),
                            stop=(p == NPAIR - 1),
                            perf_mode=DROW,
                            skip_group_check=True,
                        )
                        for ch in range(CH):
                            nc.tensor.matmul(
                                av[ch][:],
                                vtp_ap[:, :, ch * 128:(ch + 1) * 128],
                                ptp_ap,
                                start=(p == 0),
                                stop=(p == NPAIR - 1),
                                perf_mode=DROW,
                                skip_group_check=True,
                            )

                    def warm(k):
                        # PE pstate warmup burst into the energy psum ring:
                        # keeps the array streaming so the projections and
                        # first energy groups run at full clock
                        wt = psE.tile([128, 2 * IBLK], F32,
                                      name=f"warm_{k}", tag="et", bufs=2)
                        nc.tensor.matmul(
                            wt[:, 0:256], ones_sb[:, 0:128], ones_sb[:],
                            start=True, stop=True,
                        )

                    if n == 0:
                        for w in range(NWARM):
                            warm(w)
                        k_chunk(0)
                        for w in range(4):
                            warm(NWARM + w)
                        q_block(0)
                    for g in range(NG):
                        if n == 0 and g == LAG:
                            loan[0] = False
                        et_group(n, g, pt)
                        kq = fill_kq.get((n, g))
                        seq = [kq] if kq else []
                        pairs = []
                        if n > 0 and g < LAG:
                            # spilled av groups of the previous block
                            gg = NG - LAG + g
                            pairs = [(2 * gg, lambda f=prev_av, p=2 * gg: f(p)),
                                     (2 * gg + 1,
                                      lambda f=prev_av, p=2 * gg + 1: f(p))]
                        elif g >= LAG:
                            gg = g - LAG
                            pairs = [(2 * gg, lambda p=2 * gg: av_pair(p)),
                                     (2 * gg + 1,
                                      lambda p=2 * gg + 1: av_pair(p))]
                        vleft = [(j, (lambda j=j: vt_pair(j)))
                                 for j in fill_vt.get((n, g), ())]
                        # weave vt pairs between av pairs so each scratch
                        # bank's drain is covered by non-psa matmul work;
                        # an av pair's own vt pair always emits before it
                        for p, pth in pairs:
                            seq += [th for j, th in vleft if j == p]
                            vleft = [(j, th) for j, th in vleft if j != p]
                            seq.append(pth)
                            if vleft:
                                seq.append(vleft.pop(0)[1])
                        seq += [th for j, th in vleft]
                        for thunk in seq:
                            thunk()
                        if n > 0 and g == LAG - 1:
                            prev_tail()
                    prev_av = av_pair
                    prev_tail = make_tail(n, get_avden, last=(n == NIB - 1))
                for p in range(2 * (NG - LAG), 2 * NG):
                    prev_av(p)
                prev_tail()
    nc.compile()
    return nc


_NC_CACHE = {}


def kernel(x, y, Wq, bq, Wk, bk, Wv, bv, gamma):
    assert x.shape == (B, C, 64, 64)
    xs = np.ascontiguousarray(x.reshape(B, C, HW).astype(np.float32))
    ys = np.ascontiguousarray(y.reshape(B, C, HW).astype(np.float32))
    wq4 = np.ascontiguousarray(np.tile(Wq.T.astype(np.float32), (1, 4)))
    wk4 = np.ascontiguousarray(np.tile(Wk.T.astype(np.float32), (1, 4)))
    wvT = np.ascontiguousarray(Wv.T.astype(np.float32))
    bq4 = np.ascontiguousarray(np.tile(bq.astype(np.float32), 4).reshape(128, 1))
    bk4 = np.ascontiguousarray(np.tile(bk.astype(np.float32), 4).reshape(128, 1))
    g = float(np.asarray(gamma).reshape(-1)[0])
    gmh = np.full((128, 1), g, dtype=np.float32)

    if "nc" not in _NC_CACHE:
        _NC_CACHE["nc"] = _build()
    nc = _NC_CACHE["nc"]

    in_maps = [
        {
            "xr": xs[b], "yr": ys[b],
            "wq4": wq4, "wk4": wk4, "wvT": wvT,
            "bq4": bq4, "bk4": bk4, "gmd": gmh,
        }
        for b in range(B)
    ]
    res = run_bass_kernel_spmd(nc, in_maps, list(range(B)))
    outs = np.stack([res.results[b]["out"] for b in range(B)])
    # gamma*bv is a per-channel constant shift of the output; the device
    # kernel computes gamma*AV/den + x, so apply it host-side (bv is zero
    # for this problem's inputs, making this a no-op)
    gbv = (g * bv.astype(np.float32))
    if np.any(gbv):
        outs = outs + gbv[None, :, None]
    return outs.reshape(B, C, 64, 64).astype(np.float32)


# revision 46
# speedup vs baseline: 1.0063x; 1.0063x over previous
"""Cross-modal attention kernel for Trainium2 -- data-parallel over batch on 8 cores.

Reference computation per sample (C=256, H=W=64, N=H*W=4096, dqk=32):
    q = Wq @ x + bq; k = Wk @ y + bk; v = Wv @ y + bv
    out = gamma * (v @ softmax_j(q^T k)^T) + x

The Act engine's exp over the full 4096x4096 energy matrix is the binding
resource (~133us busy at 0.833ns/col on [128, free] tiles, dtype-independent),
so the kernel is organized as one continuous act stream with everything else
scheduled underneath it:

  - No separate projection phase: q/k/v projections are interleaved into the
    attention pipeline's PE-queue slack as deadline-ordered "filler" work.
  - Wq/Wk are loaded pre-replicated 4x along the output dim, so one matmul
    per channel-chunk directly yields q/k in the 4-row-group layout that the
    4-way tile_position energy packing wants (no broadcast copies at all).
  - Energy is computed TRANSPOSED (E^T[j,i], keys on partitions); exp is
    applied unnormalized to fp8e4m3; AV and the softmax denominator are
    MatmulPerfMode.DoubleRow fp8 contractions (2 j-tiles per pass); the
    denominator's normalization happens on the [C, IBLK] output.
  - The exp act table is preloaded at t=0 by a dummy 1-element activation;
    a burst of warmup matmuls keeps PE continuously busy from t~0.4us so it
    reaches full pstate before the first projection.
  - AV runs at lag 3 behind the energy groups and the last three AV groups
    of each block spill into the next block's first three slots, so the PE
    queue always has ready work and its idle gaps stay below the pstate
    reset threshold.
  - The residual x is re-read from the f32r SBUF projection operand via
    bitcast -- x is DMA'd once, not twice.
  - Tail (1/den, *gamma, +x, store) is split DVE/Pool so the two channel
    chunks normalize in parallel; within each AV pair the den matmul issues
    first so the final block's tail starts as early as possible.

PSUM budget (8 banks of 2KB): energy ring 2x[128,1024]f32 = 4, av accums
2x[128,512] = 2, den 1, projection scratch [128,512] = 1.
"""

import sys

if "/opt/trn_rl_repo" not in sys.path:
    sys.path.insert(0, "/opt/trn_rl_repo")

import numpy as np

import concourse.bacc as bacc
import concourse.mybir as mybir
import concourse.tile as tile
from concourse.bass_utils import run_bass_kernel_spmd

F32 = mybir.dt.float32
F32R = mybir.dt.float32r
BF16 = mybir.dt.bfloat16
FP8 = mybir.dt.float8e4

B, C, HW, D = 8, 256, 4096, 32
CH = C // 128
IBLK = 512
NIB = HW // IBLK
NJT = HW // 128
NPAIR = NJT // 2
NG = NJT // 4
NWARM = 12
LAG = 6
EXPF = mybir.ActivationFunctionType.Exp
MULT = mybir.AluOpType.mult
ADD = mybir.AluOpType.add
DROW = mybir.MatmulPerfMode.DoubleRow


def _build():
    nc = bacc.Bacc("TRN2", target_bir_lowering=False, debug=False, num_devices=8)

    xr = nc.dram_tensor("xr", [C, HW], F32R, kind="ExternalInput")
    yr = nc.dram_tensor("yr", [C, HW], F32R, kind="ExternalInput")
    wq4d = nc.dram_tensor("wq4", [C, 128], F32R, kind="ExternalInput")
    wk4d = nc.dram_tensor("wk4", [C, 128], F32R, kind="ExternalInput")
    wvd = nc.dram_tensor("wvT", [C, C], F32R, kind="ExternalInput")
    bq4d = nc.dram_tensor("bq4", [128, 1], F32, kind="ExternalInput")
    bk4d = nc.dram_tensor("bk4", [128, 1], F32, kind="ExternalInput")
    gmd = nc.dram_tensor("gmd", [128, 1], F32, kind="ExternalInput")
    out = nc.dram_tensor("out", [C, HW], F32, kind="ExternalOutput")

    tc = tile.TileContext(nc)
    with tc:
        with (
            tc.tile_pool(name="cst", bufs=1) as cst,
            tc.tile_pool(name="io", bufs=1) as io,
            tc.tile_pool(name="qkv", bufs=1) as qkv,
        ):
            wq4_sb = cst.tile([128, CH * 128], F32R)
            wk4_sb = cst.tile([128, CH * 128], F32R)
            wv_sb = cst.tile([128, CH * C], F32R)
            bq4_sb = cst.tile([128, 1], F32)
            bk4_sb = cst.tile([128, 1], F32)
            gm_sb = cst.tile([128, 1], F32)
            ones_sb = cst.tile([128, 2 * 128], FP8)
            scr = cst.tile([1, 2], F32)

            xr_sb = io.tile([128, CH * HW], F32R)
            yr_sb = io.tile([128, CH * HW], F32R)
            q4 = qkv.tile([128, HW], BF16)
            k4 = qkv.tile([128, HW], BF16)
            vt = qkv.tile([128, NJT * C], FP8)

            # exp act-table preload: first act-engine instruction, no deps
            # beyond the memset, so LoadActFuncSet runs at t~0.  memset on
            # Pool (95ns) so the PE warmup below can start immediately.
            nc.gpsimd.memset(ones_sb[:], 1.0)
            nc.scalar.activation(scr[:, 0:1], ones_sb[0:1, 0:1], EXPF)

            # weights + small constants via the gpsimd SWDGE queue: its
            # descriptor generation runs in parallel with the HWDGE input
            # stream, and the transfers are small enough to slip between
            # the big input transfers on the DMA engines
            nc.gpsimd.dma_start(
                wk4_sb[:].rearrange("P (ch c) -> P ch c", ch=CH),
                wk4d.rearrange("(ch p) c -> p ch c", ch=CH),
            )
            nc.gpsimd.dma_start(bk4_sb[:], bk4d[:])
            nc.gpsimd.dma_start(gm_sb[:], gmd[:])

            # inputs on the sync/HWDGE queue in dependency-deadline order:
            # the first energy group needs wk4+y0 then wq4+x0; y chunk c
            # feeds k-chunk c (needed at block-0 slot c) and vt chunk c.
            yr3 = yr.rearrange("(ch p) N -> p ch N", ch=CH)
            xr3 = xr.rearrange("(ch p) N -> p ch N", ch=CH)
            yr_sb3 = yr_sb[:].rearrange("P (ch N) -> P ch N", ch=CH)
            xr_sb3 = xr_sb[:].rearrange("P (ch N) -> P ch N", ch=CH)

            def ld(dst3, src3, c0, c1):
                nc.sync.dma_start(dst3[:, :, c0:c1], src3[:, :, c0:c1])

            ld(yr_sb3, yr3, 0, IBLK)
            nc.sync.dma_start(wq4_sb[:, 0:128], wq4d[0:128, :])
            nc.sync.dma_start(wq4_sb[:, 128:256], wq4d[128:256, :])
            # x chunk 0 split per channel-chunk so q0's two matmuls can
            # pipeline with the two transfers; bq4 after x0 (only the
            # bias-add needs it)
            nc.sync.dma_start(xr_sb[:, 0:IBLK], xr[0:128, 0:IBLK])
            nc.sync.dma_start(xr_sb[:, HW:HW + IBLK], xr[128:256, 0:IBLK])
            nc.sync.dma_start(bq4_sb[:], bq4d[:])
            ld(yr_sb3, yr3, IBLK, 2 * IBLK)
            ld(yr_sb3, yr3, 2 * IBLK, 3 * IBLK)
            nc.sync.dma_start(wv_sb[:, 0:C], wvd[0:128, :])
            nc.sync.dma_start(wv_sb[:, C:2 * C], wvd[128:256, :])
            for c in range(3, NIB):
                ld(yr_sb3, yr3, c * IBLK, (c + 1) * IBLK)
            for n in range(1, NIB):
                ld(xr_sb3, xr3, n * IBLK, (n + 1) * IBLK)

            with (
                tc.tile_pool(name="ptp", bufs=2) as ptp,
                tc.tile_pool(name="wrk", bufs=2) as wrk,
                tc.tile_pool(name="psE", bufs=1, space="PSUM") as psE,
                tc.tile_pool(name="psAV", bufs=1, space="PSUM") as psAV,
                tc.tile_pool(name="psPR", bufs=1, space="PSUM") as psPR,
            ):
                psa = psPR.tile([128, IBLK], F32, name="psa")
                vrr = [0]
                kln = [0]
                loan = [True]

                def vt_region():
                    # scratch bank for a vt j-tile PAIR.  While the av/den
                    # psum banks are still unused (before block 0's first av
                    # accumulation) they are loaned out for a depth-3
                    # rotation; afterwards the single psa bank serves the
                    # few remaining pairs.
                    if loan[0]:
                        r = vrr[0] & 1
                        vrr[0] += 1
                        if r == 0:
                            return psa[:]
                        t = psAV.tile([128, IBLK], F32,
                                      name=f"vln_{vrr[0]}", tag="av0")
                        return t[:]
                    return psa[:]

                def kq_region():
                    if loan[0]:
                        kln[0] += 1
                        t = psAV.tile([128, IBLK], F32,
                                      name=f"kln_{kln[0]}",
                                      tag="den" if kln[0] & 1 else "av1")
                        return t[:]
                    return psa[:]

                def k_chunk(c, w_sb=None, b_sb=None, src=None, dst=None):
                    w_sb = wk4_sb if w_sb is None else w_sb
                    b_sb = bk4_sb if b_sb is None else b_sb
                    src = yr_sb if src is None else src
                    dst = k4 if dst is None else dst
                    ps = kq_region()
                    for h in range(CH):
                        nc.tensor.matmul(
                            ps,
                            w_sb[:, h * 128:(h + 1) * 128],
                            src[:, h * HW + c * IBLK: h * HW + (c + 1) * IBLK],
                            start=(h == 0),
                            stop=(h == CH - 1),
                        )
                    nc.vector.tensor_scalar_add(
                        dst[:, c * IBLK:(c + 1) * IBLK], ps, b_sb[:, 0:1]
                    )

                def q_block(n):
                    k_chunk(n, wq4_sb, bq4_sb, xr_sb, q4)

                def vt_pair(p):
                    # project two j-tiles into one scratch bank, drain with
                    # a single [128,512] DVE copy (Pool cannot read PSUM on
                    # real hw, so all drains share DVE -- pairing halves the
                    # per-tile drain cost)
                    ps = vt_region()
                    for s in range(2):
                        jt = 2 * p + s
                        for h in range(CH):
                            nc.tensor.matmul(
                                ps[:, s * C:(s + 1) * C],
                                yr_sb[:, h * HW + jt * 128: h * HW + (jt + 1) * 128],
                                wv_sb[:, h * C:(h + 1) * C],
                                start=(h == 0),
                                stop=(h == CH - 1),
                            )
                    nc.vector.tensor_copy(vt[:, 2 * p * C:(2 * p + 2) * C], ps[:])

                # deadline-ordered projection filler, split into vt tiles
                # (interleaved between other PE work so the psum scratch
                # ping-pong latency hides) and k/q chunks.
                fill_vt = {}
                fill_kq = {}
                for g in range(6):
                    # loan window: 2 vt pairs per slot on the loaned scratch
                    fill_vt[(0, g)] = [2 * g, 2 * g + 1]
                fill_vt[(1, 0)] = [14]
                fill_vt[(1, 1)] = [15]
                fill_vt[(1, 2)] = [12]
                fill_vt[(1, 3)] = [13]
                for g in range(1, 8):
                    fill_kq[(0, g - 1)] = lambda g=g: k_chunk(g)
                fill_kq[(0, 7)] = lambda: q_block(1)
                fill_kq[(1, 5)] = lambda: q_block(2)
                for n in range(2, 7):
                    fill_kq[(n, 1)] = lambda n=n: q_block(n + 1)

                def et_group(n, g, pt):
                    # energy for (i-block n, group g): 4 row-packed K=32
                    # matmuls into two 2-bank psum tiles, then exp into pt
                    ets = [
                        psE.tile([128, 2 * IBLK], F32,
                                 name=f"et{h}_{n}_{g}", tag="et", bufs=2)
                        for h in range(2)
                    ]
                    for q in range(4):
                        jt = 4 * g + q
                        nc.tensor.matmul(
                            ets[q // 2][:, (q % 2) * IBLK:(q % 2 + 1) * IBLK],
                            k4[32 * q:32 * (q + 1), jt * 128:(jt + 1) * 128],
                            q4[32 * q:32 * (q + 1), n * IBLK:(n + 1) * IBLK],
                            start=True,
                            stop=True,
                            tile_position=(32 * q, 0),
                        )
                    for h in range(2):
                        nc.scalar.activation(
                            pt[:, (4 * g + 2 * h) * IBLK:(4 * g + 2 * h + 2) * IBLK],
                            ets[h][:], EXPF,
                        )

                ones_pair = ones_sb[:].rearrange("P (s c) -> P s c", s=2)

                def make_tail(n, get_avden, last=False):
                    def tail():
                        av, den = get_avden()
                        rgb = wrk.tile([128, IBLK], F32,
                                       name=f"rgb_{n}", tag="rgb")
                        nc.vector.reciprocal(rgb[:], den[:])
                        ot = wrk.tile([128, CH * IBLK], F32, name=f"ot_{n}", tag="ot")
                        # gamma folds into the scalar slot of the stt, so the
                        # whole tail is reciprocal + 2 ops per channel (all
                        # DVE: av is PSUM, which only DVE can read)
                        for ch in range(CH):
                            xres = xr_sb[
                                :, ch * HW + n * IBLK: ch * HW + (n + 1) * IBLK
                            ].bitcast(F32)
                            tmp = wrk.tile([128, IBLK], F32,
                                           name=f"tmp_{n}_{ch}", tag=f"tmp{ch}")
                            nc.vector.scalar_tensor_tensor(
                                tmp[:], av[ch][:], gm_sb[:, 0:1], rgb[:],
                                MULT, MULT,
                            )
                            # the final +x is SBUF-only, so ch0's can run on
                            # Pool while DVE continues with ch1's stt
                            eng = nc.gpsimd if ch == 0 else nc.vector
                            eng.tensor_tensor(
                                ot[:, ch * IBLK:(ch + 1) * IBLK],
                                tmp[:], xres, ADD,
                            )
                            # per-channel store so ch0 ships while ch1 runs
                            nc.sync.dma_start(
                                out[ch * 128:(ch + 1) * 128,
                                    n * IBLK:(n + 1) * IBLK],
                                ot[:, ch * IBLK:(ch + 1) * IBLK],
                            )
                    return tail

                prev_av = None
                prev_tail = None
                for n in range(NIB):
                    pt = ptp.tile([128, NJT * IBLK], FP8, name=f"pt_{n}", tag="pt")

                    # av/den accumulators are allocated lazily at the first
                    # av_pair so block 0's loan tiles (same tags) precede
                    # them in the ring's WAR chain
                    holder = {}

                    def get_avden(n=n, holder=holder):
                        if "av" not in holder:
                            holder["av"] = [
                                psAV.tile([128, IBLK], F32,
                                          name=f"av{ch}_{n}", tag=f"av{ch}")
                                for ch in range(CH)
                            ]
                            holder["den"] = psAV.tile(
                                [128, IBLK], F32, name=f"den_{n}", tag="den")
                        return holder["av"], holder["den"]

                    def av_pair(p, pt=pt, get=get_avden):
                        av, den = get()
                        # DoubleRow AV + denominator for j-tile pair p:
                        # virtual K=256 contracts two j-tiles at once.  den
                        # first so the last block's tail can start before
                        # its final av matmuls retire.
                        ptp_ap = pt[:, 2 * p * IBLK:(2 * p + 2) * IBLK].rearrange(
                            "P (s N) -> P s N", s=2
                        )
                        vtp_ap = vt[:, 2 * p * C:(2 * p + 2) * C].rearrange(
                            "P (s c) -> P s c", s=2
                        )
                        nc.tensor.matmul(
                            den[:],
                            ones_pair,
                            ptp_ap,
                            start=(p == 0),
                            stop=(p == NPAIR - 1),
                            perf_mode=DROW,
                            skip_group_check=True,
                        )
                        for ch in range(CH):
                            nc.tensor.matmul(
                                av[ch][:],
                                vtp_ap[:, :, ch * 128:(ch + 1) * 128],
                                ptp_ap,
                                start=(p == 0),
                                stop=(p == NPAIR - 1),
                                perf_mode=DROW,
                                skip_group_check=True,
                            )

                    def warm(k):
                        # PE pstate warmup burst into the energy psum ring:
                        # keeps the array streaming so the projections and
                        # first energy groups run at full clock
                        wt = psE.tile([128, 2 * IBLK], F32,
                                      name=f"warm_{k}", tag="et", bufs=2)
                        nc.tensor.matmul(
                            wt[:, 0:256], ones_sb[:, 0:128], ones_sb[:],
                            start=True, stop=True,
                        )

                    if n == 0:
                        for w in range(NWARM):
                            warm(w)
                        k_chunk(0)
                        for w in range(4):
                            warm(NWARM + w)
                        q_block(0)
                    for g in range(NG):
                        if n == 0 and g == LAG:
                            loan[0] = False
                        et_group(n, g, pt)
                        kq = fill_kq.get((n, g))
                        seq = [kq] if kq else []
                        pairs = []
                        if n > 0 and g < LAG:
                            # spilled av groups of the previous block
                            gg = NG - LAG + g
                            pairs = [(2 * gg, lambda f=prev_av, p=2 * gg: f(p)),
                                     (2 * gg + 1,
                                      lambda f=prev_av, p=2 * gg + 1: f(p))]
                        elif g >= LAG:
                            gg = g - LAG
                            pairs = [(2 * gg, lambda p=2 * gg: av_pair(p)),
                                     (2 * gg + 1,
                                      lambda p=2 * gg + 1: av_pair(p))]
                        vleft = [(j, (lambda j=j: vt_pair(j)))
                                 for j in fill_vt.get((n, g), ())]
                        # weave vt pairs between av pairs so each scratch
                        # bank's drain is covered by non-psa matmul work;
                        # an av pair's own vt pair always emits before it
                        for p, pth in pairs:
                            seq += [th for j, th in vleft if j == p]
                            vleft = [(j, th) for j, th in vleft if j != p]
                            seq.append(pth)
                            if vleft:
                                seq.append(vleft.pop(0)[1])
                        seq += [th for j, th in vleft]
                        for thunk in seq:
                            thunk()
                        if n > 0 and g == LAG - 1:
                            prev_tail()
                    prev_av = av_pair
                    prev_tail = make_tail(n, get_avden, last=(n == NIB - 1))
                for p in range(2 * (NG - LAG), 2 * NG):
                    prev_av(p)
                prev_tail()
    nc.compile()
    return nc


_NC_CACHE = {}


def kernel(x, y, Wq, bq, Wk, bk, Wv, bv, gamma):
    assert x.shape == (B, C, 64, 64)
    xs = np.ascontiguousarray(x.reshape(B, C, HW).astype(np.float32))
    ys = np.ascontiguousarray(y.reshape(B, C, HW).astype(np.float32))
    wq4 = np.ascontiguousarray(np.tile(Wq.T.astype(np.float32), (1, 4)))
    wk4 = np.ascontiguousarray(np.tile(Wk.T.astype(np.float32), (1, 4)))
    wvT = np.ascontiguousarray(Wv.T.astype(np.float32))
    bq4 = np.ascontiguousarray(np.tile(bq.astype(np.float32), 4).reshape(128, 1))
    bk4 = np.ascontiguousarray(np.tile(bk.astype(np.float32), 4).reshape(128, 1))
    g = float(np.asarray(gamma).reshape(-1)[0])
    gmh = np.full((128, 1), g, dtype=np.float32)

    if "nc" not in _NC_CACHE:
        _NC_CACHE["nc"] = _build()
    nc = _NC_CACHE["nc"]

    in_maps = [
        {
            "xr": xs[b], "yr": ys[b],
            "wq4": wq4, "wk4": wk4, "wvT": wvT,
            "bq4": bq4, "bk4": bk4, "gmd": gmh,
        }
        for b in range(B)
    ]
    res = run_bass_kernel_spmd(nc, in_maps, list(range(B)))
    outs = np.stack([res.results[b]["out"] for b in range(B)])
    # gamma*bv is a per-channel constant shift of the output; the device
    # kernel computes gamma*AV/den + x, so apply it host-side (bv is zero
    # for this problem's inputs, making this a no-op)
    gbv = (g * bv.astype(np.float32))
    if np.any(gbv):
        outs = outs + gbv[None, :, None]
    return outs.reshape(B, C, 64, 64).astype(np.float32)


# revision 47
# speedup vs baseline: 1.0108x; 1.0044x over previous
"""Cross-modal attention kernel for Trainium2 -- data-parallel over batch on 8 cores.

Reference computation per sample (C=256, H=W=64, N=H*W=4096, dqk=32):
    q = Wq @ x + bq; k = Wk @ y + bk; v = Wv @ y + bv
    out = gamma * (v @ softmax_j(q^T k)^T) + x

The Act engine's exp over the full 4096x4096 energy matrix is the binding
resource (~133us busy at 0.833ns/col on [128, free] tiles, dtype-independent),
so the kernel is organized as one continuous act stream with everything else
scheduled underneath it:

  - No separate projection phase: q/k/v projections are interleaved into the
    attention pipeline's PE-queue slack as deadline-ordered "filler" work.
  - Wq/Wk are loaded pre-replicated 4x along the output dim, so one matmul
    per channel-chunk directly yields q/k in the 4-row-group layout that the
    4-way tile_position energy packing wants (no broadcast copies at all).
  - Energy is computed TRANSPOSED (E^T[j,i], keys on partitions); exp is
    applied unnormalized to fp8e4m3; AV and the softmax denominator are
    MatmulPerfMode.DoubleRow fp8 contractions (2 j-tiles per pass); the
    denominator's normalization happens on the [C, IBLK] output.
  - The exp act table is preloaded at t=0 by a dummy 1-element activation;
    a burst of warmup matmuls keeps PE continuously busy from t~0.4us so it
    reaches full pstate before the first projection.
  - AV runs at lag 3 behind the energy groups and the last three AV groups
    of each block spill into the next block's first three slots, so the PE
    queue always has ready work and its idle gaps stay below the pstate
    reset threshold.
  - The residual x is re-read from the f32r SBUF projection operand via
    bitcast -- x is DMA'd once, not twice.
  - Tail (1/den, *gamma, +x, store) is split DVE/Pool so the two channel
    chunks normalize in parallel; within each AV pair the den matmul issues
    first so the final block's tail starts as early as possible.

PSUM budget (8 banks of 2KB): energy ring 2x[128,1024]f32 = 4, av accums
2x[128,512] = 2, den 1, projection scratch [128,512] = 1.
"""

import sys

if "/opt/trn_rl_repo" not in sys.path:
    sys.path.insert(0, "/opt/trn_rl_repo")

import numpy as np

import concourse.bacc as bacc
import concourse.mybir as mybir
import concourse.tile as tile
from concourse.bass_utils import run_bass_kernel_spmd

F32 = mybir.dt.float32
F32R = mybir.dt.float32r
BF16 = mybir.dt.bfloat16
FP8 = mybir.dt.float8e4

B, C, HW, D = 8, 256, 4096, 32
CH = C // 128
IBLK = 512
NIB = HW // IBLK
NJT = HW // 128
NPAIR = NJT // 2
NG = NJT // 4
NWARM = 12
LAG = 6
EXPF = mybir.ActivationFunctionType.Exp
MULT = mybir.AluOpType.mult
ADD = mybir.AluOpType.add
DROW = mybir.MatmulPerfMode.DoubleRow


def _build():
    nc = bacc.Bacc("TRN2", target_bir_lowering=False, debug=False, num_devices=8)

    xr = nc.dram_tensor("xr", [C, HW], F32R, kind="ExternalInput")
    yr = nc.dram_tensor("yr", [C, HW], F32R, kind="ExternalInput")
    wq4d = nc.dram_tensor("wq4", [C, 128], F32R, kind="ExternalInput")
    wk4d = nc.dram_tensor("wk4", [C, 128], F32R, kind="ExternalInput")
    wvd = nc.dram_tensor("wvT", [C, C], F32R, kind="ExternalInput")
    bq4d = nc.dram_tensor("bq4", [128, 1], F32, kind="ExternalInput")
    bk4d = nc.dram_tensor("bk4", [128, 1], F32, kind="ExternalInput")
    gmd = nc.dram_tensor("gmd", [128, 1], F32, kind="ExternalInput")
    out = nc.dram_tensor("out", [C, HW], F32, kind="ExternalOutput")

    tc = tile.TileContext(nc)
    with tc:
        with (
            tc.tile_pool(name="cst", bufs=1) as cst,
            tc.tile_pool(name="io", bufs=1) as io,
            tc.tile_pool(name="qkv", bufs=1) as qkv,
        ):
            wq4_sb = cst.tile([128, CH * 128], F32R)
            wk4_sb = cst.tile([128, CH * 128], F32R)
            wv_sb = cst.tile([128, CH * C], F32R)
            bq4_sb = cst.tile([128, 1], F32)
            bk4_sb = cst.tile([128, 1], F32)
            gm_sb = cst.tile([128, 1], F32)
            ones_sb = cst.tile([128, 2 * 128], FP8)
            scr = cst.tile([1, 2], F32)

            xr_sb = io.tile([128, CH * HW], F32R)
            yr_sb = io.tile([128, CH * HW], F32R)
            q4 = qkv.tile([128, HW], BF16)
            k4 = qkv.tile([128, HW], BF16)
            vt = qkv.tile([128, NJT * C], FP8)

            # exp act-table preload: first act-engine instruction, no deps
            # beyond the memset, so LoadActFuncSet runs at t~0.  memset on
            # Pool (95ns) so the PE warmup below can start immediately.
            nc.gpsimd.memset(ones_sb[:], 1.0)
            nc.scalar.activation(scr[:, 0:1], ones_sb[0:1, 0:1], EXPF)

            # weights + small constants via the gpsimd SWDGE queue: its
            # descriptor generation runs in parallel with the HWDGE input
            # stream, and the transfers are small enough to slip between
            # the big input transfers on the DMA engines
            nc.gpsimd.dma_start(
                wk4_sb[:].rearrange("P (ch c) -> P ch c", ch=CH),
                wk4d.rearrange("(ch p) c -> p ch c", ch=CH),
            )
            nc.gpsimd.dma_start(bk4_sb[:], bk4d[:])
            nc.gpsimd.dma_start(gm_sb[:], gmd[:])

            # inputs on the sync/HWDGE queue in dependency-deadline order:
            # the first energy group needs wk4+y0 then wq4+x0; y chunk c
            # feeds k-chunk c (needed at block-0 slot c) and vt chunk c.
            yr3 = yr.rearrange("(ch p) N -> p ch N", ch=CH)
            xr3 = xr.rearrange("(ch p) N -> p ch N", ch=CH)
            yr_sb3 = yr_sb[:].rearrange("P (ch N) -> P ch N", ch=CH)
            xr_sb3 = xr_sb[:].rearrange("P (ch N) -> P ch N", ch=CH)

            def ld(dst3, src3, c0, c1):
                nc.sync.dma_start(dst3[:, :, c0:c1], src3[:, :, c0:c1])

            ld(yr_sb3, yr3, 0, IBLK)
            nc.sync.dma_start(wq4_sb[:, 0:128], wq4d[0:128, :])
            nc.sync.dma_start(wq4_sb[:, 128:256], wq4d[128:256, :])
            # x chunk 0 split per channel-chunk so q0's two matmuls can
            # pipeline with the two transfers; bq4 after x0 (only the
            # bias-add needs it)
            nc.sync.dma_start(xr_sb[:, 0:IBLK], xr[0:128, 0:IBLK])
            nc.sync.dma_start(xr_sb[:, HW:HW + IBLK], xr[128:256, 0:IBLK])
            nc.sync.dma_start(bq4_sb[:], bq4d[:])
            ld(yr_sb3, yr3, IBLK, 2 * IBLK)
            nc.sync.dma_start(wv_sb[:, 0:C], wvd[0:128, :])
            nc.sync.dma_start(wv_sb[:, C:2 * C], wvd[128:256, :])
            for c in range(2, NIB):
                ld(yr_sb3, yr3, c * IBLK, (c + 1) * IBLK)
            for n in range(1, NIB):
                ld(xr_sb3, xr3, n * IBLK, (n + 1) * IBLK)

            with (
                tc.tile_pool(name="ptp", bufs=2) as ptp,
                tc.tile_pool(name="wrk", bufs=2) as wrk,
                tc.tile_pool(name="psE", bufs=1, space="PSUM") as psE,
                tc.tile_pool(name="psAV", bufs=1, space="PSUM") as psAV,
                tc.tile_pool(name="psPR", bufs=1, space="PSUM") as psPR,
            ):
                psa = psPR.tile([128, IBLK], F32, name="psa")
                vrr = [0]
                kln = [0]
                loan = [True]

                def vt_region():
                    # scratch bank for a vt j-tile PAIR.  While the av/den
                    # psum banks are still unused (before block 0's first av
                    # accumulation) they are loaned out for a depth-3
                    # rotation; afterwards the single psa bank serves the
                    # few remaining pairs.
                    if loan[0]:
                        r = vrr[0] & 1
                        vrr[0] += 1
                        if r == 0:
                            return psa[:]
                        t = psAV.tile([128, IBLK], F32,
                                      name=f"vln_{vrr[0]}", tag="av0")
                        return t[:]
                    return psa[:]

                def kq_region():
                    if loan[0]:
                        kln[0] += 1
                        t = psAV.tile([128, IBLK], F32,
                                      name=f"kln_{kln[0]}",
                                      tag="den" if kln[0] & 1 else "av1")
                        return t[:]
                    return psa[:]

                def k_chunk(c, w_sb=None, b_sb=None, src=None, dst=None):
                    w_sb = wk4_sb if w_sb is None else w_sb
                    b_sb = bk4_sb if b_sb is None else b_sb
                    src = yr_sb if src is None else src
                    dst = k4 if dst is None else dst
                    ps = kq_region()
                    for h in range(CH):
                        nc.tensor.matmul(
                            ps,
                            w_sb[:, h * 128:(h + 1) * 128],
                            src[:, h * HW + c * IBLK: h * HW + (c + 1) * IBLK],
                            start=(h == 0),
                            stop=(h == CH - 1),
                        )
                    nc.vector.tensor_scalar_add(
                        dst[:, c * IBLK:(c + 1) * IBLK], ps, b_sb[:, 0:1]
                    )

                def q_block(n):
                    k_chunk(n, wq4_sb, bq4_sb, xr_sb, q4)

                def vt_pair(p):
                    # project two j-tiles into one scratch bank, drain with
                    # a single [128,512] DVE copy (Pool cannot read PSUM on
                    # real hw, so all drains share DVE -- pairing halves the
                    # per-tile drain cost)
                    ps = vt_region()
                    for s in range(2):
                        jt = 2 * p + s
                        for h in range(CH):
                            nc.tensor.matmul(
                                ps[:, s * C:(s + 1) * C],
                                yr_sb[:, h * HW + jt * 128: h * HW + (jt + 1) * 128],
                                wv_sb[:, h * C:(h + 1) * C],
                                start=(h == 0),
                                stop=(h == CH - 1),
                            )
                    nc.vector.tensor_copy(vt[:, 2 * p * C:(2 * p + 2) * C], ps[:])

                # deadline-ordered projection filler, split into vt tiles
                # (interleaved between other PE work so the psum scratch
                # ping-pong latency hides) and k/q chunks.
                fill_vt = {}
                fill_kq = {}
                for g in range(6):
                    # loan window: 2 vt pairs per slot on the loaned scratch
                    fill_vt[(0, g)] = [2 * g, 2 * g + 1]
                fill_vt[(1, 0)] = [14]
                fill_vt[(1, 1)] = [15]
                fill_vt[(1, 2)] = [12]
                fill_vt[(1, 3)] = [13]
                for g in range(1, 8):
                    fill_kq[(0, g - 1)] = lambda g=g: k_chunk(g)
                fill_kq[(0, 7)] = lambda: q_block(1)
                fill_kq[(1, 5)] = lambda: q_block(2)
                for n in range(2, 7):
                    fill_kq[(n, 1)] = lambda n=n: q_block(n + 1)

                def et_group(n, g, pt):
                    # energy for (i-block n, group g): 4 row-packed K=32
                    # matmuls into two 2-bank psum tiles, then exp into pt
                    ets = [
                        psE.tile([128, 2 * IBLK], F32,
                                 name=f"et{h}_{n}_{g}", tag="et", bufs=2)
                        for h in range(2)
                    ]
                    for q in range(4):
                        jt = 4 * g + q
                        nc.tensor.matmul(
                            ets[q // 2][:, (q % 2) * IBLK:(q % 2 + 1) * IBLK],
                            k4[32 * q:32 * (q + 1), jt * 128:(jt + 1) * 128],
                            q4[32 * q:32 * (q + 1), n * IBLK:(n + 1) * IBLK],
                            start=True,
                            stop=True,
                            tile_position=(32 * q, 0),
                        )
                    for h in range(2):
                        nc.scalar.activation(
                            pt[:, (4 * g + 2 * h) * IBLK:(4 * g + 2 * h + 2) * IBLK],
                            ets[h][:], EXPF,
                        )

                ones_pair = ones_sb[:].rearrange("P (s c) -> P s c", s=2)

                def make_tail(n, get_avden, last=False):
                    def tail():
                        av, den = get_avden()
                        rgb = wrk.tile([128, IBLK], F32,
                                       name=f"rgb_{n}", tag="rgb")
                        nc.vector.reciprocal(rgb[:], den[:])
                        ot = wrk.tile([128, CH * IBLK], F32, name=f"ot_{n}", tag="ot")
                        # gamma folds into the scalar slot of the stt, so the
                        # whole tail is reciprocal + 2 ops per channel (all
                        # DVE: av is PSUM, which only DVE can read)
                        for ch in range(CH):
                            xres = xr_sb[
                                :, ch * HW + n * IBLK: ch * HW + (n + 1) * IBLK
                            ].bitcast(F32)
                            tmp = wrk.tile([128, IBLK], F32,
                                           name=f"tmp_{n}_{ch}", tag=f"tmp{ch}")
                            nc.vector.scalar_tensor_tensor(
                                tmp[:], av[ch][:], gm_sb[:, 0:1], rgb[:],
                                MULT, MULT,
                            )
                            # the final +x is SBUF-only, so ch0's can run on
                            # Pool while DVE continues with ch1's stt
                            eng = nc.gpsimd if ch == 0 else nc.vector
                            eng.tensor_tensor(
                                ot[:, ch * IBLK:(ch + 1) * IBLK],
                                tmp[:], xres, ADD,
                            )
                            # per-channel store so ch0 ships while ch1 runs
                            nc.sync.dma_start(
                                out[ch * 128:(ch + 1) * 128,
                                    n * IBLK:(n + 1) * IBLK],
                                ot[:, ch * IBLK:(ch + 1) * IBLK],
                            )
                    return tail

                prev_av = None
                prev_tail = None
                for n in range(NIB):
                    pt = ptp.tile([128, NJT * IBLK], FP8, name=f"pt_{n}", tag="pt")

                    # av/den accumulators are allocated lazily at the first
                    # av_pair so block 0's loan tiles (same tags) precede
                    # them in the ring's WAR chain
                    holder = {}

                    def get_avden(n=n, holder=holder):
                        if "av" not in holder:
                            holder["av"] = [
                                psAV.tile([128, IBLK], F32,
                                          name=f"av{ch}_{n}", tag=f"av{ch}")
                                for ch in range(CH)
                            ]
                            holder["den"] = psAV.tile(
                                [128, IBLK], F32, name=f"den_{n}", tag="den")
                        return holder["av"], holder["den"]

                    def av_pair(p, pt=pt, get=get_avden):
                        av, den = get()
                        # DoubleRow AV + denominator for j-tile pair p:
                        # virtual K=256 contracts two j-tiles at once.  den
                        # first so the last block's tail can start before
                        # its final av matmuls retire.
                        ptp_ap = pt[:, 2 * p * IBLK:(2 * p + 2) * IBLK].rearrange(
                            "P (s N) -> P s N", s=2
                        )
                        vtp_ap = vt[:, 2 * p * C:(2 * p + 2) * C].rearrange(
                            "P (s c) -> P s c", s=2
                        )
                        nc.tensor.matmul(
                            den[:],
                            ones_pair,
                            ptp_ap,
                            start=(p == 0),
                            stop=(p == NPAIR - 1),
                            perf_mode=DROW,
                            skip_group_check=True,
                        )
                        for ch in range(CH):
                            nc.tensor.matmul(
                                av[ch][:],
                                vtp_ap[:, :, ch * 128:(ch + 1) * 128],
                                ptp_ap,
                                start=(p == 0),
                                stop=(p == NPAIR - 1),
                                perf_mode=DROW,
                                skip_group_check=True,
                            )

                    def warm(k):
                        # PE pstate warmup burst into the energy psum ring:
                        # keeps the array streaming so the projections and
                        # first energy groups run at full clock
                        wt = psE.tile([128, 2 * IBLK], F32,
                                      name=f"warm_{k}", tag="et", bufs=2)
                        nc.tensor.matmul(
                            wt[:, 0:256], ones_sb[:, 0:128], ones_sb[:],
                            start=True, stop=True,
                        )

                    if n == 0:
                        for w in range(NWARM):
                            warm(w)
                        k_chunk(0)
                        for w in range(4):
                            warm(NWARM + w)
                        q_block(0)
                    for g in range(NG):
                        if n == 0 and g == LAG:
                            loan[0] = False
                        et_group(n, g, pt)
                        kq = fill_kq.get((n, g))
                        seq = [kq] if kq else []
                        pairs = []
                        if n > 0 and g < LAG:
                            # spilled av groups of the previous block
                            gg = NG - LAG + g
                            pairs = [(2 * gg, lambda f=prev_av, p=2 * gg: f(p)),
                                     (2 * gg + 1,
                                      lambda f=prev_av, p=2 * gg + 1: f(p))]
                        elif g >= LAG:
                            gg = g - LAG
                            pairs = [(2 * gg, lambda p=2 * gg: av_pair(p)),
                                     (2 * gg + 1,
                                      lambda p=2 * gg + 1: av_pair(p))]
                        vleft = [(j, (lambda j=j: vt_pair(j)))
                                 for j in fill_vt.get((n, g), ())]
                        # weave vt pairs between av pairs so each scratch
                        # bank's drain is covered by non-psa matmul work;
                        # an av pair's own vt pair always emits before it
                        for p, pth in pairs:
                            seq += [th for j, th in vleft if j == p]
                            vleft = [(j, th) for j, th in vleft if j != p]
                            seq.append(pth)
                            if vleft:
                                seq.append(vleft.pop(0)[1])
                        seq += [th for j, th in vleft]
                        for thunk in seq:
                            thunk()
                        if n > 0 and g == LAG - 1:
                            prev_tail()
                    prev_av = av_pair
                    prev_tail = make_tail(n, get_avden, last=(n == NIB - 1))
                for p in range(2 * (NG - LAG), 2 * NG):
                    prev_av(p)
                prev_tail()
    nc.compile()
    return nc


_NC_CACHE = {}


def kernel(x, y, Wq, bq, Wk, bk, Wv, bv, gamma):
    assert x.shape == (B, C, 64, 64)
    xs = np.ascontiguousarray(x.reshape(B, C, HW).astype(np.float32))
    ys = np.ascontiguousarray(y.reshape(B, C, HW).astype(np.float32))
    wq4 = np.ascontiguousarray(np.tile(Wq.T.astype(np.float32), (1, 4)))
    wk4 = np.ascontiguousarray(np.tile(Wk.T.astype(np.float32), (1, 4)))
    wvT = np.ascontiguousarray(Wv.T.astype(np.float32))
    bq4 = np.ascontiguousarray(np.tile(bq.astype(np.float32), 4).reshape(128, 1))
    bk4 = np.ascontiguousarray(np.tile(bk.astype(np.float32), 4).reshape(128, 1))
    g = float(np.asarray(gamma).reshape(-1)[0])
    gmh = np.full((128, 1), g, dtype=np.float32)

    if "nc" not in _NC_CACHE:
        _NC_CACHE["nc"] = _build()
    nc = _NC_CACHE["nc"]

    in_maps = [
        {
            "xr": xs[b], "yr": ys[b],
            "wq4": wq4, "wk4": wk4, "wvT": wvT,
            "bq4": bq4, "bk4": bk4, "gmd": gmh,
        }
        for b in range(B)
    ]
    res = run_bass_kernel_spmd(nc, in_maps, list(range(B)))
    outs = np.stack([res.results[b]["out"] for b in range(B)])
    # gamma*bv is a per-channel constant shift of the output; the device
    # kernel computes gamma*AV/den + x, so apply it host-side (bv is zero
    # for this problem's inputs, making this a no-op)
    gbv = (g * bv.astype(np.float32))
    if np.any(gbv):
        outs = outs + gbv[None, :, None]
    return outs.reshape(B, C, 64, 64).astype(np.float32)
